# revision 9
# baseline (speedup 1.0000x reference)
"""Trainium2 Bass kernel for a single-layer decoder-only transformer.

Problem shapes: B=2, S=2048, D=1024, M=4096, OMEGA=32000 (fp32 reference).

Sharding: 8 cores; core c owns the 512-query chunk q0=(c%4)*512 of batch
b=c//4 (token-parallel).  Every core runs the identical SPMD program:

  A. gather emb rows (bf16) for its batch's 2048 context tokens (+ its 512
     query tokens), add positional encoding, transpose to feature-major h^T.
  B. K^T = Wk^T h^T (kept in SBUF), V = h^T^T Wv (token-major),
     Q^T = Wq^T hq^T.
  C. causal attention for the 512 queries over the full 2048-key context
     (host-provided additive masks make causality data-driven so the program
     is uniform across cores), then Wo projection and the gelu FFN, all kept
     feature-major: ffT [4096, 512].
  D. logits = ffT^T @ Wl + bl, streamed over the full 32000-col vocab;
     each core writes its own [512, 32000] fp32 output slab.

All matmuls run in bf16 with fp32 PSUM accumulation.  PSUM is drained
exclusively by the Scalar engine: concurrent Vector-engine PSUM reads
were measured to slow the PE matmul issue pace from 216 ns to 259 ns
per 512-wide matmul, so per-column bias adds happen SBUF-side on the
Vector engine after an Activation-engine PSUM->SBUF copy.
"""

import numpy as np
import ml_dtypes

import concourse.bass as bass
import concourse.bacc as bacc
import concourse.tile as tile
import concourse.mybir as mybir
from concourse.bass_utils import run_bass_kernel_spmd
from concourse.masks import make_identity

P = 128
B, S, D, M, V = 2, 2048, 1024, 4096, 32000
TQ = 512              # queries per core
CTX = S               # context length (uniform across cores)
NCORES = 8
DC = D // P           # 8 feature chunks
MC = M // P           # 32 ffn chunks
KC = CTX // P         # 16 key chunks
QS = TQ // P          # 4 query subtiles
N_TILE = 512
N_TILES = [(i * N_TILE, min(N_TILE, V - i * N_TILE)) for i in range((V + N_TILE - 1) // N_TILE)]

F32 = mybir.dt.float32
BF16 = mybir.dt.bfloat16
I32 = mybir.dt.int32
AF = mybir.ActivationFunctionType
AX = mybir.AxisListType

_CACHE = {}


def _bcast_ap(t, offset, n, length):
    """DRAM AP broadcasting a [length] row to [n, length] partitions."""
    return bass.AP(tensor=t.tensor, offset=offset, ap=[[0, n], [1, length]])


def build_program():
    nc = bacc.Bacc("TRN2", target_bir_lowering=False, debug=False,
                   num_devices=NCORES)

    def din(name, shape, dt):
        return nc.dram_tensor(name, shape, dt, kind="ExternalInput").ap()

    xk = din("xk", [CTX, 1], I32)
    xq = din("xq", [TQ, 1], I32)
    emb = din("emb", [V, D], BF16)
    pek = din("pek", [CTX, D], BF16)
    peq = din("peq", [TQ, D], BF16)
    wq = din("wq", [D, D], BF16)
    wk = din("wk", [D, D], BF16)
    wv = din("wv", [D, D], BF16)
    wo = din("wo", [D, D], BF16)
    wf = din("wf", [D, M], BF16)
    wl = din("wl", [M, V], BF16)
    bq = din("bq", [D], F32)
    bk = din("bk", [D], F32)
    bv = din("bv", [D], F32)
    bo = din("bo", [D], F32)
    bf_ = din("bf", [M], F32)
    bl = din("bl", [V], F32)
    maskq = din("maskq", [QS, P, CTX], BF16)
    out = nc.dram_tensor("out", [TQ, V], F32, kind="ExternalOutput").ap()

    wq_r = wq.rearrange("(c p) o -> p c o", p=P)
    wk_r = wk.rearrange("(c p) o -> p c o", p=P)
    wv_r = wv.rearrange("(c p) o -> p c o", p=P)
    wo_r = wo.rearrange("(c p) o -> p c o", p=P)
    wf_r = wf.rearrange("(c p) o -> p c o", p=P)
    wl_r = wl.rearrange("(c p) o -> p c o", p=P)

    with tile.TileContext(nc) as tc:
        _emit(nc, tc, locals())
    nc.compile()
    return nc


def _emit(nc, tc, t):
    import contextlib
    ctx = contextlib.ExitStack()
    with ctx:
        main = ctx.enter_context(tc.tile_pool(name="main", bufs=1))
        poolC = ctx.enter_context(tc.tile_pool(name="poolC", bufs=1))
        poolW = ctx.enter_context(tc.tile_pool(name="poolW", bufs=2))
        stAB = ctx.enter_context(contextlib.ExitStack())
        poolAB = stAB.enter_context(tc.tile_pool(name="poolAB", bufs=1))
        # PSUM pools (own LIFO stack)
        stPB = ctx.enter_context(contextlib.ExitStack())
        psB = stPB.enter_context(tc.tile_pool(name="psB", bufs=2, space="PSUM"))
        stPA = stPB.enter_context(contextlib.ExitStack())
        psA = stPA.enter_context(tc.tile_pool(name="psA", bufs=2, space="PSUM"))

        ident = main.tile([P, P], BF16, tag="ident")
        make_identity(nc, ident[:])

        # ---- biases ----
        bqt = main.tile([P, DC], F32, tag="bqt")
        bkt = main.tile([P, DC], F32, tag="bkt")
        bot = main.tile([P, DC], F32, tag="bot")
        bft = main.tile([P, MC], F32, tag="bft")
        bvb = main.tile([P, D], BF16, tag="bvb")
        nc.sync.dma_start(out=bqt[:], in_=t["bq"].rearrange("(c p) -> p c", p=P))
        nc.sync.dma_start(out=bkt[:], in_=t["bk"].rearrange("(c p) -> p c", p=P))
        nc.sync.dma_start(out=bot[:], in_=t["bo"].rearrange("(c p) -> p c", p=P))
        nc.sync.dma_start(out=bft[:], in_=t["bf_"].rearrange("(c p) -> p c", p=P))
        nc.gpsimd.dma_start(out=bvb[:], in_=_bcast_ap(t["bv"], 0, P, D))

        hT = poolAB.tile([P, DC, CTX], BF16, tag="hT")    # h^T  [D, CTX]
        hqT = poolAB.tile([P, DC, TQ], BF16, tag="hqT")   # hq^T [D, TQ]
        qT = poolAB.tile([P, DC, TQ], BF16, tag="qT")     # Q^T  [D, TQ]
        kT = poolAB.tile([P, DC, CTX], BF16, tag="kT")    # K^T  [D, CTX]
        vtok = poolAB.tile([P, KC, D], BF16, tag="vtok")  # V    [CTX, D]

        # ---- stage A: embedding gather + positional add + transpose ----
        # (query groups first so stage-B Q can start early)
        with nc.named_scope("embed"), tc.tile_pool(name="ga", bufs=3) as ga:
            def embed_group(ids_dram, pe_dram, g, dstT):
                idx = ga.tile([P, 1], I32, tag="idx")
                nc.sync.dma_start(out=idx[:], in_=ids_dram[g * P:(g + 1) * P, :])
                hrow = ga.tile([P, D], BF16, tag="hrow")
                nc.gpsimd.indirect_dma_start(
                    out=hrow[:], out_offset=None, in_=t["emb"][:, :],
                    in_offset=bass.IndirectOffsetOnAxis(ap=idx[:, :1], axis=0))
                pet = ga.tile([P, D], BF16, tag="pet")
                nc.sync.dma_start(out=pet[:], in_=pe_dram[g * P:(g + 1) * P, :])
                nc.vector.tensor_add(hrow[:], hrow[:], pet[:])
                for dc in range(DC):
                    pt_ = psA.tile([P, P], BF16, tag="psT", space="PSUM")
                    nc.tensor.transpose(out=pt_[:], in_=hrow[:, dc * P:(dc + 1) * P],
                                        identity=ident[:])
                    nc.scalar.activation(dstT[:, dc, g * P:(g + 1) * P], pt_[:],
                                         AF.Copy)

            for g in range(QS):
                embed_group(t["xq"], t["peq"], g, hqT)
            for g in range(KC):
                embed_group(t["xk"], t["pek"], g, hT)

        # ---- stage B: projections ----
        # d->d weights stream through one 8KB half-tile tag (bufs=2) so the
        # next half prefetches while the current one is consumed.
        def load_w_half(wr, h):
            w_ = poolW.tile([P, DC // 2, D], BF16, tag="w_h")
            nc.sync.dma_start(out=w_[:], in_=wr[:, h * 4:(h + 1) * 4, :])
            return w_

        def acc_halves(ps_ap, halves, col, rhs_fn):
            for h in (0, 1):
                for dj in range(4):
                    di = h * 4 + dj
                    nc.tensor.matmul(out=ps_ap,
                                     lhsT=halves[h][:, dj, col],
                                     rhs=rhs_fn(di),
                                     start=(di == 0), stop=(di == DC - 1))

        with nc.named_scope("qkv"):
            wq_h = [load_w_half(t["wq_r"], h) for h in (0, 1)]
            for dc in range(DC):
                ps = psB.tile([P, N_TILE], F32, tag="psB", space="PSUM")
                acc_halves(ps[:, :TQ], wq_h, slice(dc * P, (dc + 1) * P),
                           lambda di: hqT[:, di, :])
                nc.scalar.activation(qT[:, dc, :], ps[:, :TQ], AF.Identity,
                                     bias=bqt[:, dc:dc + 1])

            wk_h = [load_w_half(t["wk_r"], h) for h in (0, 1)]
            for tc4 in range(CTX // N_TILE):
                sl = slice(tc4 * N_TILE, (tc4 + 1) * N_TILE)
                for dc in range(DC):
                    ps = psB.tile([P, N_TILE], F32, tag="psB", space="PSUM")
                    acc_halves(ps[:], wk_h, slice(dc * P, (dc + 1) * P),
                               lambda di: hT[:, di, sl])
                    nc.scalar.activation(kT[:, dc, sl], ps[:], AF.Identity,
                                         bias=bkt[:, dc:dc + 1])

            wv_h = [load_w_half(t["wv_r"], h) for h in (0, 1)]
            for kc in range(KC):
                for nn in range(D // N_TILE):
                    sl = slice(nn * N_TILE, (nn + 1) * N_TILE)
                    ps = psB.tile([P, N_TILE], F32, tag="psB", space="PSUM")
                    for h in (0, 1):
                        for dj in range(4):
                            di = h * 4 + dj
                            nc.tensor.matmul(
                                out=ps[:],
                                lhsT=hT[:, di, kc * P:(kc + 1) * P],
                                rhs=wv_h[h][:, dj, sl],
                                start=(di == 0), stop=(di == DC - 1))
                    # ACT drains PSUM; bias add stays SBUF-side on DVE
                    nc.scalar.activation(vtok[:, kc, sl], ps[:], AF.Copy)
                    nc.vector.tensor_add(vtok[:, kc, sl], vtok[:, kc, sl],
                                         bvb[:, sl])

        # ---- stage C1: scores + softmax + transpose to P^T ----
        pT = poolC.tile([P, KC, TQ], BF16, tag="pT")      # P^T  [CTX, TQ]
        atT = poolC.tile([P, DC, TQ], BF16, tag="atT")    # attn^T
        aoT = poolC.tile([P, DC, TQ], BF16, tag="aoT")    # (attn Wo)^T

        inv_sqrt_d = 1.0 / float(np.sqrt(D))
        with nc.named_scope("attn"):
            with tc.tile_pool(name="sm", bufs=2) as sm, \
                 tc.tile_pool(name="psS", bufs=3, space="PSUM") as psS:
                s_sb = [sm.tile([P, CTX], BF16, tag=f"s_sb{m}", name=f"s_sb{m}",
                                bufs=1) for m in range(QS)]
                for kb in range(CTX // N_TILE):
                    sl = slice(kb * N_TILE, (kb + 1) * N_TILE)
                    for m in range(QS):
                        ps = psS.tile([P, N_TILE], F32, tag="psS", space="PSUM")
                        for di in range(DC):
                            nc.tensor.matmul(out=ps[:],
                                             lhsT=qT[:, di, m * P:(m + 1) * P],
                                             rhs=kT[:, di, sl],
                                             start=(di == 0), stop=(di == DC - 1))
                        nc.scalar.activation(s_sb[m][:, sl], ps[:], AF.Copy)
                for m in range(QS):
                    # additive causal mask (0 visible / -1e9 masked); scores
                    # are O(10) pre-scale so exp needs no max-subtraction.
                    mask = sm.tile([P, CTX], BF16, tag="mask")
                    nc.sync.dma_start(out=mask[:], in_=t["maskq"][m, :, :])
                    nc.vector.tensor_add(s_sb[m][:], s_sb[m][:], mask[:])
                    den = sm.tile([P, 1], F32, tag="den")
                    nc.scalar.activation(s_sb[m][:], s_sb[m][:], AF.Exp,
                                         scale=inv_sqrt_d,
                                         accum_out=den[:, :1])
                    rden = sm.tile([P, 1], F32, tag="rden")
                    nc.vector.reciprocal(rden[:], den[:])
                    nc.vector.tensor_scalar_mul(s_sb[m][:], s_sb[m][:],
                                                rden[:, :1])
                    for kc in range(KC):
                        pt_ = psA.tile([P, P], BF16, tag="psT", space="PSUM")
                        nc.tensor.transpose(out=pt_[:],
                                            in_=s_sb[m][:, kc * P:(kc + 1) * P],
                                            identity=ident[:])
                        nc.scalar.activation(pT[:, kc, m * P:(m + 1) * P], pt_[:],
                                             AF.Copy)
            stPA.close()  # transpose psum done after C1

            # ---- stage C2: attn^T = V-blocks^T @ P^T ----
            for dc in range(DC):
                ps = psB.tile([P, N_TILE], F32, tag="psB", space="PSUM")
                for kc in range(KC):
                    nc.tensor.matmul(out=ps[:, :TQ],
                                     lhsT=vtok[:, kc, dc * P:(dc + 1) * P],
                                     rhs=pT[:, kc, :],
                                     start=(kc == 0), stop=(kc == KC - 1))
                nc.scalar.activation(atT[:, dc, :], ps[:, :TQ], AF.Copy)
            stAB.close()  # hT/hqT/qT/kT/vtok dead after C2

            # stage-D pools open here so Wl slab prefetch can fill the
            # C3a/C3b DMA-idle window (space freed by poolAB)
            poolFF = ctx.enter_context(tc.tile_pool(name="poolFF", bufs=1))
            wlp = ctx.enter_context(tc.tile_pool(name="wlp", bufs=1))
            blp = ctx.enter_context(tc.tile_pool(name="blp", bufs=3))
            outp = ctx.enter_context(tc.tile_pool(name="outp", bufs=6))

            # ---- stage C3a: attnout^T = Wo^T @ attn^T ----
            wo_h = [load_w_half(t["wo_r"], h) for h in (0, 1)]
            for dc in range(DC):
                ps = psB.tile([P, N_TILE], F32, tag="psB", space="PSUM")
                acc_halves(ps[:, :TQ], wo_h, slice(dc * P, (dc + 1) * P),
                           lambda di: atT[:, di, :])
                nc.scalar.activation(aoT[:, dc, :], ps[:, :TQ], AF.Identity,
                                     bias=bot[:, dc:dc + 1])

        # ---- stage C3b: ffT = gelu(Wf^T @ aoT + bf) ----
        ffT = poolFF.tile([P, MC, TQ], BF16, tag="ffT")   # ff^T [M, TQ]
        with nc.named_scope("ffn"), tc.tile_pool(name="poolWF", bufs=2) as poolWF:
            # wf streamed in M-column quarters: quarter q serves mc 8q..8q+7
            MQ = MC // 4
            for q in range(4):
                w_ = poolWF.tile([P, DC, MQ * P], BF16, tag="w_f")
                nc.sync.dma_start(
                    out=w_[:], in_=t["wf_r"][:, :, q * MQ * P:(q + 1) * MQ * P])
                for mj in range(MQ):
                    mc = q * MQ + mj
                    ps = psB.tile([P, N_TILE], F32, tag="psB", space="PSUM")
                    for di in range(DC):
                        nc.tensor.matmul(out=ps[:, :TQ],
                                         lhsT=w_[:, di, mj * P:(mj + 1) * P],
                                         rhs=aoT[:, di, :],
                                         start=(di == 0), stop=(di == DC - 1))
                    nc.scalar.activation(ffT[:, mc, :], ps[:, :TQ], AF.Gelu,
                                         bias=bft[:, mc:mc + 1])
        stPB.close()

        # ---- stage D: logits = ffT^T @ Wl + bl ----
        # PSUM is drained by the Scalar engine only (a concurrent Vector
        # PSUM read slows the PE stream 216 -> 259 ns/matmul); the bl bias
        # add runs SBUF-side on the Vector engine.
        with nc.named_scope("logits"), \
             tc.tile_pool(name="psD", bufs=6, space="PSUM") as psD:
            for (n0, nsz) in N_TILES:
                slab = wlp.tile([P, MC, N_TILE], BF16, tag="slab", bufs=2)
                for qq in range(4):
                    nc.sync.dma_start(
                        out=slab[:, qq * 8:(qq + 1) * 8, :nsz],
                        in_=t["wl_r"][:, qq * 8:(qq + 1) * 8, n0:n0 + nsz])
                blt = blp.tile([P, N_TILE], F32, tag="blt")
                nc.gpsimd.dma_start(out=blt[:, :nsz],
                                    in_=_bcast_ap(t["bl"], n0, P, nsz))
                for m in range(QS):
                    ps = psD.tile([P, N_TILE], F32, tag="psD", space="PSUM")
                    for kc in range(MC):
                        nc.tensor.matmul(out=ps[:, :nsz],
                                         lhsT=ffT[:, kc, m * P:(m + 1) * P],
                                         rhs=slab[:, kc, :nsz],
                                         start=(kc == 0), stop=(kc == MC - 1))
                    ot = outp.tile([P, N_TILE], F32, tag="ot")
                    nc.scalar.activation(ot[:, :nsz], ps[:, :nsz], AF.Copy)
                    nc.vector.tensor_add(ot[:, :nsz], ot[:, :nsz], blt[:, :nsz])
                    nc.sync.dma_start(
                        out=t["out"][m * P:(m + 1) * P, n0:n0 + nsz],
                        in_=ot[:, :nsz])


def _prep_inputs(x, emb, pe, Wq, bq, Wk, bk, Wv, bv, Wo, bo, Wf, bf, Wl, bl):
    """Host-side sharding / layout prep (no data-dependent compute)."""
    bf16 = ml_dtypes.bfloat16
    x = np.asarray(x)
    shared = {
        "emb": np.ascontiguousarray(np.asarray(emb).astype(bf16)),
        "pek": np.ascontiguousarray(np.asarray(pe)[:CTX].astype(bf16)),
        "wq": np.asarray(Wq).astype(bf16),
        "wk": np.asarray(Wk).astype(bf16),
        "wv": np.asarray(Wv).astype(bf16),
        "wo": np.asarray(Wo).astype(bf16),
        "wf": np.asarray(Wf).astype(bf16),
        "wl": np.ascontiguousarray(np.asarray(Wl).astype(bf16)),
        "bq": np.asarray(bq, np.float32),
        "bk": np.asarray(bk, np.float32),
        "bv": np.asarray(bv, np.float32),
        "bo": np.asarray(bo, np.float32),
        "bf": np.asarray(bf, np.float32),
        "bl": np.asarray(bl, np.float32),
    }
    pe16 = shared["pek"]
    in_maps = []
    for c in range(NCORES):
        b, j = divmod(c, NCORES // B)
        q0 = j * TQ
        mask = np.zeros((QS, P, CTX), dtype=bf16)
        for m in range(QS):
            gq = q0 + m * P + np.arange(P)[:, None]
            vis = np.arange(CTX)[None, :] <= gq
            mask[m] = np.where(vis, np.float32(0.0), np.float32(-1e9)).astype(bf16)
        im = dict(shared)
        im["xk"] = np.ascontiguousarray(x[b].astype(np.int32).reshape(CTX, 1))
        im["xq"] = np.ascontiguousarray(
            x[b, q0:q0 + TQ].astype(np.int32).reshape(TQ, 1))
        im["peq"] = np.ascontiguousarray(pe16[q0:q0 + TQ])
        im["maskq"] = mask
        in_maps.append(im)
    return in_maps


def kernel(**inputs):
    if "nc" not in _CACHE:
        _CACHE["nc"] = build_program()
    nc = _CACHE["nc"]
    in_maps = _prep_inputs(**inputs)
    res = run_bass_kernel_spmd(nc, in_maps, list(range(NCORES)))
    x = np.asarray(inputs["x"])
    Bsz, Ssz = x.shape
    out = np.empty((Bsz, Ssz, V), np.float32)
    for c in range(NCORES):
        b, j = divmod(c, NCORES // B)
        q0 = j * TQ
        out[b, q0:q0 + TQ] = res.results[c]["out"]
    return out


if __name__ == "__main__":
    pass
